# revision 4
# baseline (speedup 1.0000x reference)
# Trainium2 Bass kernel for nn_Consolidation_24283745092289 (topk_masking).
# Self-contained: shards batch B across 8 NeuronCores (data parallel),
# runs one Bass/Tile kernel per core, gathers the full output.
import sys
sys.path.insert(0, '/opt/trn_rl_repo')
from contextlib import ExitStack
import numpy as np

import concourse.bass as bass
import concourse.mybir as mybir
import concourse.tile as tile
from concourse import bacc
from concourse.bass_utils import run_bass_kernel_spmd
from concourse.masks import make_identity

F32 = mybir.dt.float32
F16 = mybir.dt.float16
OP = mybir.AluOpType
AF = mybir.ActivationFunctionType

T, B, NQ, NKV, D = 8, 8, 1024, 1024, 512
DC = D // 128          # 4 feature chunks of 128
TOPK = T // 2          # 4
BN_EPS = 1e-5
SCALE = float(D) ** -0.5


def _build_nc():
    nc = bacc.Bacc("TRN2", target_bir_lowering=False, debug=False, num_devices=8)

    q_in = nc.dram_tensor("q", [T, NQ, D], F32, kind="ExternalInput").ap()
    kv_in = nc.dram_tensor("kv", [T, NKV, D], F32, kind="ExternalInput").ap()
    gw_in = nc.dram_tensor("gw", [D, D], F32, kind="ExternalInput").ap()
    pw_in = nc.dram_tensor("pw", [D, D], F32, kind="ExternalInput").ap()
    vecs = {}
    for name in ["gg", "gb", "gm", "gv", "pg", "pb", "pm", "pv"]:
        vecs[name] = nc.dram_tensor(name, [D], F32, kind="ExternalInput").ap()
    out_d = nc.dram_tensor("out", [T, NQ, D], F32, kind="ExternalOutput").ap()

    with tile.TileContext(nc) as tc, ExitStack() as ctx:
        per = ctx.enter_context(tc.tile_pool(name="persist", bufs=1))

        ident32 = per.tile([128, 128], F32, tag="id32")
        ident16 = per.tile([128, 128], F16, tag="id16")
        make_identity(nc, ident32[:])
        make_identity(nc, ident16[:])

        # ---- weight transposes: W [e, d] -> WT [d, e] (both [512, 512]) ----
        WgT = per.tile([128, DC, D], F32, tag="WgT")
        WpT = per.tile([128, DC, D], F32, tag="WpT")
        with ExitStack() as sctx:
            wld = sctx.enter_context(tc.tile_pool(name="wld", bufs=2))
            wps = sctx.enter_context(tc.tile_pool(name="wps", bufs=2, space="PSUM"))
            for (win, WT) in ((gw_in, WgT), (pw_in, WpT)):
                wt = wld.tile([128, DC, D], F32, tag="w")
                nc.sync.dma_start(wt[:], win.rearrange("(i p) d -> p i d", p=128))
                for dc in range(DC):
                    ps = wps.tile([128, 512], F32, tag="ps")
                    for i in range(4):
                        nc.tensor.transpose(ps[:, i * 128:(i + 1) * 128],
                                            wt[:, i, dc * 128:(dc + 1) * 128], ident32[:])
                    nc.scalar.copy(WT[:, dc, :], ps[:])

            # ---- BN affine constants (e on partitions, [128, DC]) ----
            def bn_consts(g, b, m, v, extra_scale):
                tg = wld.tile([128, DC], F32, tag="bn_g")
                tb = wld.tile([128, DC], F32, tag="bn_b")
                tm = wld.tile([128, DC], F32, tag="bn_m")
                tv = wld.tile([128, DC], F32, tag="bn_v")
                for t_, src in ((tg, g), (tb, b), (tm, m), (tv, v)):
                    nc.sync.dma_start(t_[:], src.rearrange("(c p) -> p c", p=128))
                rs = per.tile([128, DC], F32, tag="bn_tmp")
                nc.vector.tensor_scalar_add(rs[:], tv[:], BN_EPS)
                nc.vector.reciprocal(rs[:], rs[:])
                nc.scalar.sqrt(rs[:], rs[:])            # rsqrt(var + eps)
                sc = per.tile([128, DC], F32, tag=f"sc{extra_scale}")
                bi = per.tile([128, DC], F32, tag=f"bi{extra_scale}")
                nc.vector.tensor_mul(sc[:], tg[:], rs[:])          # gamma * rsqrt
                nc.vector.tensor_mul(rs[:], tm[:], sc[:])          # rmean * s
                nc.vector.tensor_sub(bi[:], tb[:], rs[:])          # beta - rmean*s
                nc.vector.tensor_scalar_mul(bi[:], bi[:], 0.5)     # LIF 1/tau fold
                nc.vector.tensor_scalar_mul(sc[:], sc[:], 0.5 * extra_scale)
                return sc, bi

            sc_g, bi_g = bn_consts(vecs["gg"], vecs["gb"], vecs["gm"], vecs["gv"], 1.0)
            sc_p, bi_p = bn_consts(vecs["pg"], vecs["pb"], vecs["pm"], vecs["pv"], SCALE)

        # ---- persistent state ----
        gT = per.tile([128, DC, NKV], F16, tag="gT")      # g^T [e, n] exact fp16
        g_nf = per.tile([128, 8, D], F16, tag="g_nf")     # g [n, e]
        v2 = per.tile([128, DC, NQ], F32, tag="v2")       # proj LIF state [e, qi]
        nc.gpsimd.memset(v2[:], 0.0)

        # ================= STAGE 1: gate linear + BN + LIF -> g =================
        with ExitStack() as sctx:
            vst = sctx.enter_context(tc.tile_pool(name="vst", bufs=1))
            v_g = vst.tile([128, DC, NKV], F32, tag="v_g")
            gacc = vst.tile([128, DC, NKV], F32, tag="gacc")
            nc.gpsimd.memset(v_g[:], 0.0)
            nc.gpsimd.memset(gacc[:], 0.0)

            kvp = sctx.enter_context(tc.tile_pool(name="kvp", bufs=2))
            kvtp = sctx.enter_context(tc.tile_pool(name="kvtp", bufs=2))
            yhp = sctx.enter_context(tc.tile_pool(name="yhp", bufs=4))
            hp = sctx.enter_context(tc.tile_pool(name="hp", bufs=2))
            ps1 = sctx.enter_context(tc.tile_pool(name="ps1", bufs=2, space="PSUM"))
            ps2 = sctx.enter_context(tc.tile_pool(name="ps2", bufs=4, space="PSUM"))

            for t in range(T):
                for nb in range(2):
                    n0 = nb * 512
                    kv = kvp.tile([128, 4, 512], F32, tag="kv")
                    nc.sync.dma_start(
                        kv[:], kv_in[t, n0:n0 + 512, :].rearrange("(r p) d -> p r d", p=128))
                    kvT = kvtp.tile([128, DC, 512], F32, tag="kvT")
                    for dc in range(DC):
                        ps = ps1.tile([128, 512], F32, tag="kvtps")
                        for r in range(4):
                            nc.tensor.transpose(ps[:, r * 128:(r + 1) * 128],
                                                kv[:, r, dc * 128:(dc + 1) * 128], ident32[:])
                        nc.scalar.copy(kvT[:, dc, :], ps[:])
                    for ec in range(DC):
                        yp = ps2.tile([128, 512], F32, tag="yps")
                        for dc in range(DC):
                            nc.tensor.matmul(yp[:], WgT[:, dc, ec * 128:(ec + 1) * 128],
                                             kvT[:, dc, :], start=(dc == 0), stop=(dc == DC - 1))
                        yh = yhp.tile([128, 512], F32, tag="yh")
                        nc.scalar.activation(yh[:], yp[:], AF.Identity,
                                             bias=bi_g[:, ec:ec + 1], scale=sc_g[:, ec:ec + 1])
                        vs = v_g[:, ec, n0:n0 + 512]
                        ga = gacc[:, ec, n0:n0 + 512]
                        h = hp.tile([128, 512], F32, tag="h")
                        nc.vector.scalar_tensor_tensor(h[:], vs, 0.5, yh[:],
                                                       op0=OP.mult, op1=OP.add)
                        nc.vector.scalar_tensor_tensor(ga, h[:], 1.0, ga,
                                                       op0=OP.is_lt, op1=OP.add)
                        nc.vector.scalar_tensor_tensor(vs, h[:], 1.0, h[:],
                                                       op0=OP.is_lt, op1=OP.mult)

            # g^T = 1 - gacc/8  (exact fp16), then transpose to g [n, e]
            for ec in range(DC):
                nc.scalar.activation(gT[:, ec, :], gacc[:, ec, :], AF.Identity,
                                     bias=1.0, scale=-0.125)
            for j in range(8):
                ps = ps1.tile([128, 512], F16, tag="gtps")
                for ec in range(DC):
                    nc.tensor.transpose(ps[:, ec * 128:(ec + 1) * 128],
                                        gT[:, ec, j * 128:(j + 1) * 128], ident16[:])
                nc.scalar.copy(g_nf[:, j, :], ps[:])

        # ========== STAGE 2: A = q@g^T, top-4 mask, update, proj, LIF ==========
        with ExitStack() as sctx:
            qld = sctx.enter_context(tc.tile_pool(name="qld", bufs=2))
            qsp = sctx.enter_context(tc.tile_pool(name="qsp", bufs=2))
            qts = sctx.enter_context(tc.tile_pool(name="qts", bufs=2))
            asb = sctx.enter_context(tc.tile_pool(name="asb", bufs=2))
            amp = sctx.enter_context(tc.tile_pool(name="amp", bufs=2))
            amt = sctx.enter_context(tc.tile_pool(name="amt", bufs=1))
            upd = sctx.enter_context(tc.tile_pool(name="upd", bufs=2))
            y2p = sctx.enter_context(tc.tile_pool(name="y2p", bufs=2))
            osb = sctx.enter_context(tc.tile_pool(name="osb", bufs=4))
            v8p = sctx.enter_context(tc.tile_pool(name="v8p", bufs=4))
            psA = sctx.enter_context(tc.tile_pool(name="psA", bufs=3, space="PSUM"))
            psB = sctx.enter_context(tc.tile_pool(name="psB", bufs=2, space="PSUM"))

            for t in range(T):
                for qb in range(2):
                    r0 = qb * 512
                    # load + fp16 hi/lo split + transpose of q rows
                    q = qld.tile([128, 4, 512], F32, tag="q")
                    nc.sync.dma_start(
                        q[:], q_in[t, r0:r0 + 512, :].rearrange("(r p) d -> p r d", p=128))
                    qh = qsp.tile([128, 4, 512], F16, tag="qh")
                    ql = qsp.tile([128, 4, 512], F16, tag="ql")
                    for r in range(4):
                        nc.scalar.copy(qh[:, r, :], q[:, r, :])
                        nc.vector.tensor_sub(ql[:, r, :], q[:, r, :], qh[:, r, :])
                    qTh = qts.tile([128, DC, 512], F16, tag="qTh")
                    qTl = qts.tile([128, DC, 512], F16, tag="qTl")
                    for (src, dst) in ((qh, qTh), (ql, qTl)):
                        ps = psA.tile([128, DC, 512], F16, tag="big")
                        for r in range(4):
                            for dc in range(DC):
                                nc.tensor.transpose(ps[:, dc, r * 128:(r + 1) * 128],
                                                    src[:, r, dc * 128:(dc + 1) * 128],
                                                    ident16[:])
                        nc.scalar.copy(dst[:], ps[:])

                    # masked A^T accumulators [n, r] fp16 hi/lo
                    amTh = amt.tile([128, 8, 512], F16, tag="amTh")
                    amTl = amt.tile([128, 8, 512], F16, tag="amTl")

                    for r in range(4):  # 128-row sub-chunks
                        aps = psA.tile([128, 1024], F32, tag="big")
                        for half in range(2):
                            hs = half * 512
                            k = 0
                            for dc in range(DC):
                                for qT in (qTh, qTl):
                                    nc.tensor.matmul(
                                        aps[:, hs:hs + 512],
                                        qT[:, dc, r * 128:(r + 1) * 128],
                                        gT[:, dc, hs:hs + 512],
                                        start=(k == 0), stop=(k == 2 * DC - 1))
                                    k += 1
                        a_sb = asb.tile([128, 1024], F32, tag="a")
                        nc.scalar.copy(a_sb[:, 0:512], aps[:, 0:512])
                        nc.scalar.copy(a_sb[:, 512:1024], aps[:, 512:1024])
                        v8 = v8p.tile([128, 8], F32, tag="v8")
                        nc.vector.max(v8[:], a_sb[:])
                        am = amp.tile([128, 1024], F32, tag="am")
                        nc.vector.scalar_tensor_tensor(am[:], a_sb[:], v8[:, 3:4], a_sb[:],
                                                       op0=OP.is_ge, op1=OP.mult)
                        amh = amp.tile([128, 1024], F16, tag="amh")
                        aml = amp.tile([128, 1024], F16, tag="aml")
                        nc.scalar.copy(amh[:], am[:])
                        nc.vector.tensor_sub(aml[:], am[:], amh[:])
                        for (src, dst) in ((amh, amTh), (aml, amTl)):
                            ps = psB.tile([128, 8, 128], F16, tag="small")
                            for j in range(8):
                                nc.tensor.transpose(ps[:, j, :],
                                                    src[:, j * 128:(j + 1) * 128], ident16[:])
                            nc.scalar.copy(dst[:, :, r * 128:(r + 1) * 128], ps[:])

                    # update^T [d, r] fp32 = sum_n g[n,d]^T.T @ Am^T[n,r] (2-pass)
                    updT = upd.tile([128, DC, 512], F32, tag="updT")
                    for hdc in range(2):
                        ups = psA.tile([128, 2, 512], F32, tag="big")
                        for d2 in range(2):
                            dc = hdc * 2 + d2
                            k = 0
                            for j in range(8):
                                for amT in (amTh, amTl):
                                    nc.tensor.matmul(
                                        ups[:, d2, :],
                                        g_nf[:, j, dc * 128:(dc + 1) * 128],
                                        amT[:, j, :],
                                        start=(k == 0), stop=(k == 15))
                                    k += 1
                        nc.scalar.copy(updT[:, hdc * 2:(hdc + 1) * 2, :], ups[:])

                    # proj: y2^T [e, r] fp32, BN(+scale folds) + LIF + spikes
                    s2 = y2p.tile([128, DC, 512], F16, tag="s2")
                    for ec in range(DC):
                        yp = psB.tile([128, 512], F32, tag="small")
                        for dc in range(DC):
                            nc.tensor.matmul(yp[:], WpT[:, dc, ec * 128:(ec + 1) * 128],
                                             updT[:, dc, :], start=(dc == 0), stop=(dc == DC - 1))
                        yh2 = y2p.tile([128, 512], F32, tag="yh2")
                        nc.scalar.activation(yh2[:], yp[:], AF.Identity,
                                             bias=bi_p[:, ec:ec + 1], scale=sc_p[:, ec:ec + 1])
                        vs = v2[:, ec, r0:r0 + 512]
                        h = y2p.tile([128, 512], F32, tag="h2")
                        nc.vector.scalar_tensor_tensor(h[:], vs, 0.5, yh2[:],
                                                       op0=OP.mult, op1=OP.add)
                        nc.vector.tensor_scalar(s2[:, ec, :], h[:], 1.0, None, op0=OP.is_ge)
                        nc.vector.scalar_tensor_tensor(vs, h[:], 1.0, h[:],
                                                       op0=OP.is_lt, op1=OP.mult)

                    # transpose spikes [e, r] -> [r, e], cast fp32, store
                    ps = psA.tile([128, 4, 512], F16, tag="big")
                    for ec in range(DC):
                        for rc in range(4):
                            nc.tensor.transpose(ps[:, rc, ec * 128:(ec + 1) * 128],
                                                s2[:, ec, rc * 128:(rc + 1) * 128], ident16[:])
                    for rc in range(4):
                        o = osb.tile([128, 512], F32, tag="o")
                        nc.scalar.copy(o[:], ps[:, rc, :])
                        nc.sync.dma_start(out_d[t, r0 + rc * 128:r0 + (rc + 1) * 128, :], o[:])

    nc.compile()
    return nc


_NC = None


def kernel(**inputs):
    global _NC
    if _NC is None:
        _NC = _build_nc()
    nc = _NC
    in_maps = []
    for b in range(B):
        in_maps.append({
            "q": np.ascontiguousarray(inputs["q"][:, b]),
            "kv": np.ascontiguousarray(inputs["kv"][:, b]),
            "gw": np.asarray(inputs["gate_W"]),
            "pw": np.asarray(inputs["proj_W"]),
            "gg": np.asarray(inputs["gate_gamma"]),
            "gb": np.asarray(inputs["gate_beta"]),
            "gm": np.asarray(inputs["gate_rmean"]),
            "gv": np.asarray(inputs["gate_rvar"]),
            "pg": np.asarray(inputs["proj_gamma"]),
            "pb": np.asarray(inputs["proj_beta"]),
            "pm": np.asarray(inputs["proj_rmean"]),
            "pv": np.asarray(inputs["proj_rvar"]),
        })
    res = run_bass_kernel_spmd(nc, in_maps, core_ids=list(range(B)))
    return np.stack([res.results[b]["out"] for b in range(B)], axis=1)
